# revision 24
# baseline (speedup 1.0000x reference)
"""Bottom-Up Hidden Tree Markov Model upward pass on 8 Trainium2 NeuronCores.

Problem: complete 8-ary forest (2 trees x 299593 nodes, depth 6), C=8 hidden
states, 32 symbols, 16 independent generative models. Output: per-tree
log-likelihood (2, 16).

Sharding: core = (tree, quarter-of-tree). Each core runs the full upward pass
over its quarter (2 complete depth-1 subtrees): 65536 leaves -> 8192 -> 1024
-> 128 -> 16 -> 2 level-1 betas. Host combines the 8 level-1 betas per tree
with the tiny root step.

Key algebraic restructurings (device does all O(N) work):
  - Leaf betas depend only on (position l, symbol s): they collapse into a
    256-row table; the level-6 einsum folds into T6[(l,s),(i,g)] so the whole
    leaf level becomes one-hot(symbol) matmuls.
  - Leaf log-nu contributions become histogram-counts x log-table (counts fall
    out of the one-hot generation for free via accum_out).
  - Interior levels: blocked matmuls with block-diagonal-over-g weights
    W_l[(j,g),(i,g)]; per-node normalize via sel/broadcast matmuls; log-nu via
    one full-tile ScalarE Ln per 4-chunk pack (memset-1.0 padding makes the
    garbage rows contribute ln(1)=0 to the accumulator).
  - All one-hots are built up front in 5 wide 4x-mode is_equal ops over
    broadcast-DMA'd bf16 symbol tiles (symbols pre-packed on host so every
    matmul slice is a plain [32 x N] window).
Partition packing everywhere: p = i*16 + g  (i = hidden state, g = generator).
"""
import os
import sys

import numpy as np

if '/opt/trn_rl_repo' not in sys.path:
    sys.path.insert(0, '/opt/trn_rl_repo')

import ml_dtypes

BF16 = ml_dtypes.bfloat16

K, DEPTH, NTREE, C, MSYM, NGEN = 8, 6, 2, 8, 32, 16
STARTS = [(K ** d - 1) // (K - 1) for d in range(DEPTH + 2)]
NT = STARTS[DEPTH + 1]          # 299593 nodes per tree
CG = C * NGEN                   # 128
NQ = 4                          # quarters per tree
LEAVES_Q = (K ** DEPTH) // NQ   # 65536 leaves per core
LQ8 = LEAVES_Q // K             # 8192 level-5 nodes per core
# packed parent-symbol tile: level-6 parents as 4 row-groups x 4 col-groups,
# then the smaller levels
XPC = 2048 + 512 + 128 + 16 + 2
XPC_PAD = 2720
# per-level (parents U, chunks, ohP col offset); the 16- and 2-node levels
# (18 nodes/core, 0.003% of the forest) finish on the host with the root step
LEVELS = [
    (8192, 16, 0),
    (1024, 2, 2048),
    (128, 1, 2560),
]
N_LL_SLOTS = 4 + 1 + 1 + 1   # per-pack Ln accums + leaf slot


def _softmax64(x, axis):
    x = np.asarray(x, np.float64)
    e = np.exp(x - x.max(axis=axis, keepdims=True))
    return e / e.sum(axis=axis, keepdims=True)


def _build_tables(A, B, Pi, SP):
    """Small O(params) tables, f64 on host. Returns dict of np arrays."""
    smA = _softmax64(A, 0)            # (C,C,K,G) over parent state i
    smB = _softmax64(B, 1)            # (C,M,G) over symbols
    smPi = _softmax64(Pi, 0)          # (C,K,G)
    smSP = _softmax64(SP, 0)          # (K,G)
    Mmat = smSP[:, None, None, :] * np.transpose(smA, (2, 0, 1, 3))  # [l,i,j,g]
    pb = smPi[:, :, None, :] * smB[:, None, :, :]     # (j, l, s, g)
    nuL = pb.sum(0)                                    # (l, s, g)
    betaLeaf = pb / nuL[None]
    llLeaf = np.log(nuL)                               # (l, s, g)
    T6 = np.einsum('lijg,jlsg->lsig', Mmat, betaLeaf)  # (l,s,i,g)
    T6f = T6.reshape(K * MSYM, CG)                     # rows (l,s), cols (i,g)
    Wl = np.zeros((K, CG, CG))
    ii = np.arange(C)
    for l in range(K):
        for g in range(NGEN):
            Wl[l, ii[:, None] * NGEN + g, ii[None, :] * NGEN + g] = Mmat[l, :, :, g].T
    BT = np.transpose(smB, (1, 0, 2)).reshape(MSYM, CG)
    llLeaf_f = llLeaf.reshape(K * MSYM, NGEN)

    p = np.arange(CG)
    sel = (p[:, None] % NGEN == np.arange(NGEN)[None, :]).astype(np.float64)
    E16 = sel.T.copy()
    tabs16 = np.concatenate([
        T6f[:128],                                        # T6a   [128,128] @0
        T6f[128:],                                        # T6b   [128,128] @128
        np.concatenate([BT] * 4, axis=0),                 # BTt4  [128,128] @256
        sel,                                              # selt  [128,16]  @384
        np.concatenate([E16] * 8, axis=0),                # E16x8 [128,128] @400
        np.concatenate([Wl[l] for l in range(K)], axis=1),  # Wt [128,1024] @528
        np.ones((CG, 512)),                               # ones @1552 (PE memset)
    ], axis=1).astype(BF16)                               # [128, 2064]
    tabs32 = np.concatenate([
        (np.arange(128) % MSYM).reshape(128, 1),          # svec @0
        llLeaf_f[:128],                                   # llLA @1
        llLeaf_f[128:],                                   # llLB @17
    ], axis=1).astype(np.float32)                         # [128, 33]
    return {'tabs16': tabs16, 'tabs32': tabs32}, Mmat.astype(np.float32), np.asarray(smB, np.float32)


def _build_bass(n_iters=1):
    import concourse.bass as bass
    import concourse.bacc as bacc
    import concourse.mybir as mybir
    from concourse import tile

    f32 = mybir.dt.float32
    bf16 = mybir.dt.bfloat16
    Alu = mybir.AluOpType
    Act = mybir.ActivationFunctionType

    nc = bacc.Bacc(None, target_bir_lowering=False)

    xs_d = nc.dram_tensor('xs', [K, LQ8], bf16, kind='ExternalInput')
    xp_d = nc.dram_tensor('xp', [4, XPC_PAD], bf16, kind='ExternalInput')
    # shape-distinct dummy input per n_iters: defeats executable-cache
    # collisions between variants that share the same I/O signature
    im_d = nc.dram_tensor('itermark', [1, n_iters], mybir.dt.float32,
                          kind='ExternalInput') if n_iters > 1 else None
    tab_specs = [('tabs16', [128, 2064], bf16), ('tabs32', [128, 33], f32)]
    tab_d = {n: nc.dram_tensor(n, s, d, kind='ExternalInput') for n, s, d in tab_specs}
    beta3_d = nc.dram_tensor('beta3', [128, 128], bf16, kind='ExternalOutput')
    llsum_d = nc.dram_tensor('llsum', [128, 1], f32, kind='ExternalOutput')

    with tile.TileContext(nc) as tc:
        with (
            tc.tile_pool(name='const', bufs=1) as constp,
            tc.tile_pool(name='beta', bufs=1) as betap,
            tc.tile_pool(name='ohsrc', bufs=1) as ohsrcp,
            tc.tile_pool(name='oh', bufs=1) as ohp,
            tc.tile_pool(name='q', bufs=10) as qp,
            tc.tile_pool(name='rp', bufs=2) as rpp,
            tc.tile_pool(name='rb', bufs=3) as rbp,
            tc.tile_pool(name='bxs', bufs=4) as bxsp,
            tc.tile_pool(name='ln', bufs=2) as lnp,
            tc.tile_pool(name='acc', bufs=1) as accp,
            tc.tile_pool(name='ps_tb', bufs=2, space='PSUM') as ps_tb,
            tc.tile_pool(name='ps_bx', bufs=2, space='PSUM') as ps_bx,
            tc.tile_pool(name='ps_nu', bufs=2, space='PSUM') as ps_nu,
            tc.tile_pool(name='ps_rb', bufs=2, space='PSUM') as ps_rb,
        ):
            if im_d is not None:
                imt = constp.tile([1, n_iters], f32, name='imt', tag='imt')
                nc.sync.dma_start(imt[:], im_d[:])
            t16 = constp.tile([128, 2064], bf16, name='t16', tag='t16')
            nc.sync.dma_start(t16[:], tab_d['tabs16'][:])
            t32 = constp.tile([128, 33], f32, name='t32', tag='t32')
            nc.sync.dma_start(t32[:], tab_d['tabs32'][:])
            TC16 = {'T6a': (0, 128), 'T6b': (128, 256), 'BTt': (256, 384),
                    'selt': (384, 400), 'E16x8': (400, 528), 'Wt': (528, 1552),
                    'ones': (1552, 2064)}
            TC32 = {'svec': (0, 1), 'llLA': (1, 17), 'llLB': (17, 33)}

            def tab(name, r0=0, r1=128, c0=0, c1=None):
                base, mx = (t16, TC16[name]) if name in TC16 else (t32, TC32[name])
                lo, hi = mx
                w = hi - lo
                if c1 is None:
                    c1 = w
                return base[r0:r1, lo + c0: lo + c1]

            beta_bufs = [
                betap.tile([128, 8192], bf16, name='b5', tag='b5'),
                betap.tile([128, 1024], bf16, name='b4', tag='b4'),
                betap.tile([128, 128], bf16, name='b3', tag='b3'),
            ]
            llparts = accp.tile([128, N_LL_SLOTS], f32, name='llparts', tag='llparts')
            cnts = accp.tile([128, 8], f32, name='cnts', tag='cnts')
            llsum_sb = accp.tile([128, 1], f32, name='llsum', tag='llsum')
            cA = accp.tile([128, 1], f32, name='cA', tag='cA')
            cB = accp.tile([128, 1], f32, name='cB', tag='cB')
            ohA_src = ohsrcp.tile([128, LQ8], bf16, name='ohA_src', tag='ohA_src')
            ohB_src = ohsrcp.tile([128, LQ8], bf16, name='ohB_src', tag='ohB_src')
            ohP_src = ohsrcp.tile([128, XPC_PAD], bf16, name='ohP_src', tag='ohP_src')
            ohA = ohp.tile([128, LQ8], bf16, name='ohA', tag='ohA')
            ohB = ohp.tile([128, LQ8], bf16, name='ohB', tag='ohB')
            ohP = ohp.tile([128, XPC_PAD], bf16, name='ohP', tag='ohP')

            for _it in range(n_iters):
                nc.gpsimd.memset(llparts[:], 0.0)
                # --- build all one-hots up front; parent symbols first (the
                # first bx matmul blocks on them), leaf halves pipelined on
                # both HWDGE queues ---
                for (c0, c1) in ((0, 1024), (1024, XPC_PAD)):
                    apP = bass.AP(xp_d[:].tensor, c0, [[XPC_PAD, 4], [0, 32], [1, c1 - c0]])
                    nc.sync.dma_start(ohP_src[:, c0:c1], apP)
                    nc.vector.tensor_scalar(ohP[:, c0:c1], ohP_src[:, c0:c1],
                                            tab('svec'), None, Alu.is_equal)
                Q = LQ8 // 4
                dmae = {0: nc.scalar, 1: nc.sync, 2: nc.scalar, 3: nc.sync}
                # (tile, quarter) pairs ordered so the first slices of A and B
                # land first on both queues
                pieces = [(0, 0), (1, 0), (0, 1), (1, 1), (0, 2), (1, 2), (0, 3), (1, 3)]
                for pi, (si, h) in enumerate(pieces):
                    src_t, oh_t, goff, ci = ((ohA_src, ohA, 0, 0), (ohB_src, ohB, 4, 4))[si]
                    ap = bass.AP(xs_d[:].tensor, goff * LQ8 + h * Q,
                                 [[LQ8, 4], [0, 32], [1, Q]])
                    dmae[pi % 4].dma_start(src_t[:, h * Q:(h + 1) * Q], ap)
                    nc.vector.tensor_scalar(
                        oh_t[:, h * Q:(h + 1) * Q], src_t[:, h * Q:(h + 1) * Q],
                        tab('svec'), None, Alu.is_equal, Alu.add,
                        accum_out=cnts[:, ci + h: ci + h + 1])
                # leaf ll from histogram counts — emitted early so it fills
                # pipeline bubbles instead of extending the tail
                nc.vector.reduce_sum(cA[:], cnts[:, 0:4], axis=mybir.AxisListType.X)
                nc.vector.reduce_sum(cB[:], cnts[:, 4:8], axis=mybir.AxisListType.X)
                llf_ps = ps_rb.tile([128, 512], f32, name='llf', tag='rbp')
                nc.tensor.matmul(llf_ps[0:16, 0:1], tab('llLA'), cA[:], start=True, stop=False)
                nc.tensor.matmul(llf_ps[0:16, 0:1], tab('llLB'), cB[:], start=False, stop=True)
                nc.scalar.copy(llparts[0:16, N_LL_SLOTS - 1:N_LL_SLOTS], llf_ps[0:16, 0:1])

                slot = 0
                for lev, (U, nch, xpo) in enumerate(LEVELS):
                    N = min(512, U)
                    out_beta = beta_bufs[lev]
                    child = beta_bufs[lev - 1] if lev > 0 else None
                    bview = child[:].rearrange('p (u l) -> p u l', l=K) if lev > 0 else None
                    pack = []
                    nu_ps = None
                    for c in range(nch):
                        if lev == 0:
                            rg, cb = c % 4, (c // 4) * 512
                        elif lev == 1:
                            rg, cb = c, 0
                        else:
                            rg, cb = 0, 0
                        bx_ps = ps_bx.tile([128, N], f32, name='bx', tag='bx')
                        nc.tensor.matmul(bx_ps[:], tab('BTt', 32 * rg, 32 * rg + 32),
                                         ohP[32 * rg:32 * rg + 32, xpo + cb:xpo + cb + N],
                                         start=True, stop=True, tile_position=(32 * rg, 0))
                        tb_ps = ps_tb.tile([128, N], f32, name='tb', tag='tb')
                        if lev == 0:
                            nc.tensor.matmul(tb_ps[:], tab('T6a'), ohA[:, c * N:(c + 1) * N],
                                             start=True, stop=False)
                            nc.tensor.matmul(tb_ps[:], tab('T6b'), ohB[:, c * N:(c + 1) * N],
                                             start=False, stop=True)
                        else:
                            for l in range(K):
                                nc.tensor.matmul(
                                    tb_ps[:], tab('Wt', c0=128 * l, c1=128 * (l + 1)),
                                    bview[:, c * N:(c + 1) * N, l],
                                    start=(l == 0), stop=(l == K - 1))
                        bx_sb = bxsp.tile([128, N], bf16, name='bxs', tag='bxs')
                        nc.scalar.copy(bx_sb[:], bx_ps[:])
                        q_sb = qp.tile([128, N], bf16, name='q', tag='q')
                        nc.vector.tensor_mul(q_sb[:], tb_ps[:], bx_sb[:])
                        a = len(pack)
                        if a == 0:
                            nu_ps = ps_nu.tile([128, N], f32, name='nu', tag='nu')
                            nc.tensor.matmul(nu_ps[:], tab('ones', 0, 1, 0, 128),
                                             tab('ones', 0, 1, 0, N),
                                             start=True, stop=True)
                        nc.tensor.matmul(nu_ps[32 * a:32 * a + 16, :], tab('selt'),
                                         q_sb[:], start=True, stop=True,
                                         tile_position=(0, 32 * a))
                        pack.append((c, q_sb, 32 * a))
                        if len(pack) == 4 or c == nch - 1:
                            ln_sb = lnp.tile([128, N], bf16, name='ln', tag='ln')
                            with nc.allow_low_precision(reason='ln output unused; accum is f32'):
                                nc.scalar.activation(ln_sb[:], nu_ps[:], Act.Ln,
                                                     accum_out=llparts[:, slot:slot + 1])
                            slot += 1
                            rp_sb = rpp.tile([128, N], bf16, name='rp', tag='rp')
                            with nc.allow_low_precision(reason='bf16 recip validated in numpy'):
                                nc.vector.reciprocal(rp_sb[:], nu_ps[:])
                            for (cc, qq, poff) in pack:
                                rb_ps = ps_rb.tile([128, N], f32, name='rbp', tag='rbp')
                                nc.tensor.matmul(rb_ps[:], tab('E16x8', poff, poff + 16),
                                                 rp_sb[poff:poff + 16, :], start=True, stop=True,
                                                 tile_position=(poff, 0))
                                dst = out_beta[:, cc * N:(cc + 1) * N]
                                if lev == 0 and cc % 4 != 3:
                                    rb_sb = rbp.tile([128, N], bf16, name='rb', tag='rb')
                                    nc.scalar.copy(rb_sb[:], rb_ps[:])
                                    nc.gpsimd.tensor_tensor(dst, qq[:], rb_sb[:], Alu.mult)
                                else:
                                    nc.vector.tensor_tensor(dst, qq[:], rb_ps[:], Alu.mult)
                            pack = []

                nc.vector.reduce_sum(llsum_sb[:], llparts[:], axis=mybir.AxisListType.X)
                nc.sync.dma_start(llsum_d[:], llsum_sb[:])
                nc.sync.dma_start(beta3_d[:], beta_bufs[2][:])
    if not nc.is_finalized():
        nc.finalize()
    return nc


_BASS_CACHE = {}


def _get_bass():
    if 'nc' not in _BASS_CACHE:
        _BASS_CACHE['nc'] = _build_bass()
    return _BASS_CACHE['nc']


def _prep_in_maps(inputs):
    A = np.asarray(inputs['A']); B = np.asarray(inputs['B'])
    Pi = np.asarray(inputs['Pi']); SP = np.asarray(inputs['SP'])
    x = np.asarray(inputs['x'])

    tables, Mmat, smB = _build_tables(A, B, Pi, SP)

    in_maps = []
    for t in range(NTREE):
        base = t * NT
        for q in range(NQ):
            s6 = base + STARTS[6] + q * LEAVES_Q
            xs = x[s6: s6 + LEAVES_Q]
            xs_t = np.ascontiguousarray(xs.reshape(LQ8, K).T).astype(BF16)  # [8, 8192]
            xp_lv = []
            for d in range(5, 0, -1):
                n_d = K ** d
                s = base + STARTS[d] + q * (n_d // NQ)
                xp_lv.append(x[s: s + n_d // NQ])
            xp = np.zeros((4, XPC_PAD), np.float32)
            # level-6 parents: 4 row-groups x 4 col-groups of 512
            x6 = xp_lv[0].reshape(4, 4, 512)          # [q, a, u] with c = 4q+a
            xp[:, 0:2048] = x6.transpose(1, 0, 2).reshape(4, 2048)
            xp[0:2, 2048:2560] = xp_lv[1].reshape(2, 512)
            xp[0, 2560:2688] = xp_lv[2]
            xp[0, 2688:2704] = xp_lv[3]
            xp[0, 2704:2706] = xp_lv[4]
            m = {'xs': xs_t, 'xp': xp.astype(BF16)}
            m.update(tables)
            in_maps.append(m)
    return in_maps, Mmat, smB


def kernel(**inputs):
    from concourse.bass_utils import run_bass_kernel_spmd

    x = np.asarray(inputs['x'])
    in_maps, Mmat, smB = _prep_in_maps(inputs)

    nc = _get_bass()
    global _LAST_IN_MAPS
    _LAST_IN_MAPS = in_maps
    res = run_bass_kernel_spmd(nc, in_maps, core_ids=list(range(8)))
    results = res.results

    Mm64 = np.asarray(Mmat, np.float64)
    smB64 = np.asarray(smB, np.float64)
    out = np.zeros((NTREE, NGEN), np.float32)
    for t in range(NTREE):
        base = t * NT
        beta1 = np.zeros((K, C, NGEN))
        for q in range(NQ):
            r = results[t * NQ + q]
            out[t] += r['llsum'][:, 0].reshape(4, 32)[:, :16].sum(0)
            # beta3 [128, 128]: column u is quarter level-3 node u, p=i*16+g
            beta = np.asarray(r['beta3'], np.float64).T.reshape(128, C, NGEN)
            # two host levels: 128 -> 16 -> 2 nodes of this quarter
            for d, U in ((2, 16), (1, 2)):
                nodes = base + STARTS[d] + q * U + np.arange(U)
                bch = beta.reshape(U, K, C, NGEN)
                tb = np.einsum('uljg,lijg->uig', bch, Mm64)
                bl = tb * np.transpose(smB64[:, x[nodes]], (1, 0, 2))
                nu = bl.sum(1)
                beta = bl / nu[:, None]
                out[t] += np.log(nu).sum(0).astype(np.float32)
            beta1[2 * q: 2 * q + 2] = beta
        tb = np.einsum('lijg,ljg->ig', Mm64, beta1)
        bl = tb * smB64[:, x[base]]
        nu = bl.sum(0)
        out[t] += np.log(nu).astype(np.float32)
    return out
